# revision 66
# baseline (speedup 1.0000x reference)
"""MFA block kernel for 8 Trainium2 NeuronCores.

Full (unsharded) inputs in, full output out. Tokens (8*1024 = 8192) are
sharded across 8 cores (1024 each).  Uses the associative rewrite

    y = theta_x @ (phi_x^T @ g_x) / BN

so the (BN, BN) attention matrix is never formed.  With X_ext = [x_l | 1]
and A = x_h @ theta_w (biasless theta), A_ext = [A | 1]:

    C_ext = X_ext^T X_ext   (257x257)
    E_ext = A_ext^T A_ext   (257x257)

Both Grams are global over tokens; they are triangle-packed (two rects
each) into ONE bf16 AllReduce payload [128 x 772] — the only collective.
Post-AllReduce, with Gw = G_ext w_w precomputed in the AR shadow:

    V  = P_ext^T (C_ext Gw) / BN        (so w_y = theta_x V)
    c^T = theta_b^T V,  Ve = [V; c^T],  W1 = E_ext Ve
    sum_t w_y[t,h]   = W1[256, h]           (BatchNorm batch stats are
    sum_t w_y[t,h]^2 = sum_k Ve[k,h]*W1[k,h] computed analytically — no
                                             second AllReduce)

w_b is dropped entirely (BN output is invariant to constant shifts).
The BN scalars compute on DVE/Act while PE prefills the w_y^T GEMM.

All DMAs are contiguous; the feature-major view x_h^T and the
token-major final store are produced with tensor-engine 128x128
transposes through PSUM (4-byte transposed-AP DMAs are ~100x slower).
Load order (x_l before x_h), per-tile bf16 casts, and queue assignment
are arranged so the Gram packs are never head-of-line blocked and the
AllReduce launches as early as possible.  Big GEMM operands are bf16
(fp32 PSUM accumulation); the x_h residual add and all BN statistics
stay fp32.  Final max rel err vs fp64 reference: ~5.8e-3.
"""

import threading

import numpy as np

import concourse.tile as tile
from concourse import bacc, mybir
from concourse.bass_utils import run_bass_kernel_spmd
from concourse.masks import make_identity

FP = mybir.dt.float32
BF = mybir.dt.bfloat16
HIGH = 512
LOW = 256
B = 8
N = 1024
BN = B * N            # 8192 flattened tokens
NCORES = 8
TPC = BN // NCORES    # 1024 tokens per core
TT = TPC // 128       # 8 token tiles per core
HC = HIGH // 128      # 4 feature chunks of x_h / w_y / z
EPS = 1e-5

LOWE = LOW + 1        # 257: homogeneous low dim
PACK = 386            # 257 + 129: triangle-packed Gram rect width
PAY = 2 * PACK        # 772: AR payload width (C pack | E pack)


def build_kernel(repeats: int = 1, noar: int = 0):
    noar = 3 if noar is True else int(noar)

    nc = bacc.Bacc("TRN2", target_bir_lowering=False, debug=False,
                   num_devices=NCORES)

    x_h = nc.declare_dram_parameter("x_h", [TPC, HIGH], FP, isOutput=False)
    x_l = nc.declare_dram_parameter("x_l", [TPC, LOW], FP, isOutput=False)
    g_w = nc.declare_dram_parameter("g_w", [LOW, LOW], FP, isOutput=False)
    g_b = nc.declare_dram_parameter("g_b", [LOW], FP, isOutput=False)
    theta_w = nc.declare_dram_parameter("theta_w", [HIGH, LOW], FP, isOutput=False)
    theta_b = nc.declare_dram_parameter("theta_b", [LOW], FP, isOutput=False)
    phi_w = nc.declare_dram_parameter("phi_w", [LOW, LOW], FP, isOutput=False)
    phi_b = nc.declare_dram_parameter("phi_b", [LOW], FP, isOutput=False)
    w_w = nc.declare_dram_parameter("w_w", [LOW, HIGH], FP, isOutput=False)
    bn_gamma = nc.declare_dram_parameter("bn_gamma", [HIGH], FP, isOutput=False)
    bn_beta = nc.declare_dram_parameter("bn_beta", [HIGH], FP, isOutput=False)
    z_out = nc.declare_dram_parameter("z", [TPC, HIGH], FP, isOutput=True)

    rg = [list(range(NCORES))]

    with tile.TileContext(nc) as tc:
        with (
            tc.tile_pool(name="sb", bufs=1) as sb,
            tc.tile_pool(name="ps", bufs=1, space="PSUM") as ps,
            tc.tile_pool(name="dram", bufs=1, space="DRAM") as dram,
        ):
            # ---- constants
            eps_c = sb.tile([128, 1], FP, tag="eps_c")
            nc.vector.memset(eps_c, EPS)
            ident = sb.tile([128, 128], FP, tag="ident")
            make_identity(nc, ident)
            ident_b = sb.tile([128, 128], BF, tag="ident_b")
            nc.gpsimd.tensor_copy(ident_b[:], ident[:])
            # ones column scaled by 1/BN: the Q/S reduction matmuls then
            # produce E[x^2] and mean directly.
            ones_c = sb.tile([128, 1], FP, tag="ones_c")
            nc.vector.memset(ones_c, 1.0 / BN)
            # Touch Sqrt once up front so the activation table set that
            # contains {sqrt, copy} is resident before the critical path.
            warm = sb.tile([128, 1], FP, tag="warm")
            nc.scalar.activation(warm[:], eps_c[:],
                                 mybir.ActivationFunctionType.Sqrt)

            for _ in range(repeats):
                # ================= loads (all contiguous) =================
                # Order: theta_w (small, feeds A soon), x_l (small; its C
                # Gram finishes early), then x_h (big: finishes last either
                # way and gates the transpose->A->E chain), then post-AR
                # weights.
                thw = sb.tile([128, HC, LOW], FP, tag="thw")
                nc.sync.dma_start(thw[:], theta_w[:, :].rearrange(
                    "(ko ki) a -> ki ko a", ki=128))
                thb = sb.tile([128, LOW // 128], FP, tag="thb")
                nc.sync.dma_start(thb[:], theta_b[:].rearrange(
                    "(ko ki) -> ki ko", ki=128))
                xle = sb.tile([128, TT, LOWE], FP, tag="xle", bufs=2)
                xle_b = sb.tile([128, TT, LOWE], BF, tag="xle_b", bufs=2)
                nc.gpsimd.memset(xle_b[:, :, LOW:LOWE], 1.0)
                # two batched DMAs (half each) kill per-tile dispatch gaps;
                # casts stay per-tile so the C Gram starts ASAP.
                for h in range(2):
                    nc.sync.dma_start(
                        xle[:, h * 4:(h + 1) * 4, 0:LOW],
                        x_l[h * 512:(h + 1) * 512, :].rearrange(
                            "(i p) c -> p i c", p=128))
                for i in range(TT):
                    nc.scalar.activation(xle_b[:, i, 0:LOW], xle[:, i, 0:LOW],
                                         mybir.ActivationFunctionType.Copy)
                xh = sb.tile([128, TT, HIGH], FP, tag="xh", bufs=2)
                xh_b = sb.tile([128, TT, HIGH], BF, tag="xh_b", bufs=2)
                for i in range(TT):
                    nc.sync.dma_start(xh[:, i, :],
                                      x_h[i * 128:(i + 1) * 128, :])
                    nc.scalar.activation(xh_b[:, i, :], xh[:, i, :],
                                         mybir.ActivationFunctionType.Copy)
                # ================= bf16 casts (scalar engine) =============
                thw_b = sb.tile([128, HC, LOW], BF, tag="thw_b")
                nc.scalar.activation(thw_b[:], thw[:],
                                     mybir.ActivationFunctionType.Copy)
                thb_b = sb.tile([128, LOW // 128], BF, tag="thb_b")
                nc.scalar.activation(thb_b[:], thb[:],
                                     mybir.ActivationFunctionType.Copy)
                # ========== x_h^T transposes + A = x_h @ theta_w ==========
                # Software-pipelined on PE: transpose group i+1 runs while
                # group i's PSUM->SBUF copy lands, then A(i) consumes it.
                c_in = dram.tile([128, PAY], BF, tag="c_in")
                c_out = dram.tile([128, PAY], BF, tag="c_out")
                xht = sb.tile([128, HC, TPC], BF, tag="xht", bufs=2)
                th_tm = sb.tile([128, TT, LOWE], BF, tag="th_tm", bufs=2)
                nc.gpsimd.memset(th_tm[:, :, LOW:LOWE], 1.0)

                def xh_transpose(i):
                    tp = ps.tile([128, HC, 128], BF, tag="tpb", bufs=2)
                    for hc in range(HC):
                        nc.tensor.transpose(
                            tp[:, hc, :], xh_b[:, i, hc * 128:(hc + 1) * 128],
                            ident_b)
                    if i % 2 == 0:
                        nc.vector.tensor_copy(
                            xht[:, :, i * 128:(i + 1) * 128], tp[:])
                    else:
                        nc.scalar.activation(
                            xht[:, :, i * 128:(i + 1) * 128], tp[:],
                            mybir.ActivationFunctionType.Copy)

                def a_mm(i):
                    aps = ps.tile([128, 512], FP, tag="mm", bufs=4)
                    for k in range(HC):
                        nc.tensor.matmul(
                            aps[:, :LOW],
                            xht[:, k, i * 128:(i + 1) * 128],
                            thw_b[:, k, :],
                            start=(k == 0), stop=(k == HC - 1))
                    if i % 2 == 0:
                        nc.vector.tensor_copy(th_tm[:, i, 0:LOW],
                                              aps[:, :LOW])
                    else:
                        nc.scalar.activation(th_tm[:, i, 0:LOW], aps[:, :LOW],
                                             mybir.ActivationFunctionType.Copy)

                # ========== C Gram: X_ext^T X_ext rects (256 x 257) =======
                for mc in range(2):
                    cps = ps.tile([128, 512], FP, tag="mm", bufs=4)
                    for i in range(TT):
                        nc.tensor.matmul(
                            cps[:, :LOWE],
                            xle_b[:, i, mc * 128:(mc + 1) * 128],
                            xle_b[:, i, :],
                            start=(i == 0), stop=(i == TT - 1))
                    if mc == 0:
                        cl_b = sb.tile([128, PACK], BF, tag="cl_b")
                        nc.vector.tensor_copy(cl_b[:, 0:LOWE], cps[:, :LOWE])
                    else:
                        nc.vector.tensor_copy(cl_b[:, LOWE:PACK],
                                              cps[:, 128:LOWE])
                        nc.sync.dma_start(c_in[:, 0:PACK], cl_b[:])


                xh_transpose(0)
                for i in range(1, TT):
                    xh_transpose(i)
                    a_mm(i - 1)
                a_mm(TT - 1)

                # ========== E Gram: A_ext^T A_ext rects ===================
                for mc in range(2):
                    eps_ = ps.tile([128, 512], FP, tag="mm", bufs=4)
                    for i in range(TT):
                        nc.tensor.matmul(
                            eps_[:, :LOWE],
                            th_tm[:, i, mc * 128:(mc + 1) * 128],
                            th_tm[:, i, :],
                            start=(i == 0), stop=(i == TT - 1))
                    if mc == 0:
                        el_b = sb.tile([128, PACK], BF, tag="el_b")
                        nc.vector.tensor_copy(el_b[:, 0:LOWE], eps_[:, :LOWE])
                    else:
                        nc.vector.tensor_copy(el_b[:, LOWE:PACK],
                                              eps_[:, 128:LOWE])
                        nc.sync.dma_start(c_in[:, PACK:PAY], el_b[:])

                # ---- post-AllReduce weights: loaded after the Gram
                #      packs so they never head-of-line block them.
                gext = sb.tile([128, 3, LOW], FP, tag="gext")
                nc.sync.dma_start(gext[:, 0:2, :], g_w[:, :].rearrange(
                    "(ko ki) a -> ki ko a", ki=128))
                nc.sync.dma_start(gext[0:1, 2, :], g_b[:][None, :])
                gbc = sb.tile([128, 2], FP, tag="gbc")
                nc.sync.dma_start(gbc[:], g_b[:].rearrange(
                    "(ko ki) -> ki ko", ki=128))
                pext = sb.tile([128, 3, LOW], FP, tag="pext")
                nc.sync.dma_start(pext[:, 0:2, :], phi_w[:, :].rearrange(
                    "(ko ki) a -> ki ko a", ki=128))
                nc.sync.dma_start(pext[0:1, 2, :], phi_b[:][None, :])
                ww = sb.tile([128, LOW // 128, HIGH], FP, tag="ww")
                nc.sync.dma_start(ww[:], w_w[:, :].rearrange(
                    "(ko ki) h -> ki ko h", ki=128))
                gamma_p = sb.tile([128, HC], FP, tag="gamma_p")
                nc.sync.dma_start(gamma_p[:], bn_gamma[:].rearrange(
                    "(hc p) -> p hc", p=128))
                beta_p = sb.tile([128, HC], FP, tag="beta_p")
                nc.sync.dma_start(beta_p[:], bn_beta[:].rearrange(
                    "(hc p) -> p hc", p=128))

                gext_b = sb.tile([128, 3, LOW], BF, tag="gext_b")
                nc.scalar.activation(gext_b[:], gext[:],
                                     mybir.ActivationFunctionType.Copy)
                gbc_b = sb.tile([128, 2], BF, tag="gbc_b")
                nc.scalar.activation(gbc_b[:], gbc[:],
                                     mybir.ActivationFunctionType.Copy)
                pext_b = sb.tile([128, 3, LOW], BF, tag="pext_b")
                nc.scalar.activation(pext_b[:], pext[:],
                                     mybir.ActivationFunctionType.Copy)
                ww_b = sb.tile([128, LOW // 128, HIGH], BF, tag="ww_b")
                nc.scalar.activation(ww_b[:], ww[:],
                                     mybir.ActivationFunctionType.Copy)


                # ========== the one AllReduce =============================
                if noar:
                    nc.sync.dma_start(c_out[:, :], c_in[:, :])
                else:
                    nc.gpsimd.collective_compute(
                        "AllReduce", mybir.AluOpType.add, replica_groups=rg,
                        ins=[c_in.opt()], outs=[c_out.opt()])

                # ========== theta^T (feature-major, + bias) — during AR ===
                # tht[:, mc, t] = A^T[mc-chunk, t] + theta_b[mc-chunk]
                tht = sb.tile([128, LOW // 128, TPC], BF, tag="tht", bufs=2)
                for i in range(TT):
                    tq = ps.tile([128, HC, 128], BF, tag="tpb", bufs=2)
                    for mc in range(LOW // 128):
                        nc.tensor.transpose(
                            tq[:, mc, :],
                            th_tm[:, i, mc * 128:(mc + 1) * 128], ident_b)
                    for mc in range(LOW // 128):
                        if (i + mc) % 2 == 0:
                            nc.vector.tensor_scalar(
                                tht[:, mc, i * 128:(i + 1) * 128],
                                tq[:, mc, :], thb[:, mc:mc + 1], None,
                                mybir.AluOpType.add)
                        else:
                            nc.scalar.activation(
                                tht[:, mc, i * 128:(i + 1) * 128],
                                tq[:, mc, :],
                                mybir.ActivationFunctionType.Identity,
                                bias=thb[:, mc:mc + 1])


                # ========== Gw = G_ext @ w_w  (257 x 512) — during AR =====
                # lhsT needs G_ext^T = [g_w^T | g_b]; transpose g_w blocks.
                gt = sb.tile([128, 2, LOWE], BF, tag="gt")
                for ki in range(2):
                    gq = ps.tile([128, HC, 128], BF, tag="tpb", bufs=2)
                    for mc in range(2):
                        nc.tensor.transpose(
                            gq[:, mc, :],
                            gext_b[:, mc, ki * 128:(ki + 1) * 128], ident_b)
                    nc.vector.tensor_copy(gt[:, ki, 0:LOW],
                                          gq[:, 0:2, :])
                    nc.vector.tensor_copy(gt[:, ki, LOW:LOWE],
                                          gbc_b[:, ki:ki + 1])
                gwa = sb.tile([128, HIGH], BF, tag="gwa")
                gwb = sb.tile([128, HIGH], BF, tag="gwb")
                gwrow = sb.tile([1, HIGH], BF, tag="gwrow")
                for mc in range(3):
                    msl = (slice(0, 128), slice(128, 256),
                           slice(256, 257))[mc]
                    mlen = msl.stop - msl.start
                    gf = ps.tile([128, 512], FP, tag="mm", bufs=4)
                    gps = gf[:mlen, :]
                    for ki in range(2):
                        nc.tensor.matmul(gps, gt[:, ki, msl],
                                         ww_b[:, ki, :],
                                         start=(ki == 0), stop=(ki == 1))
                    dst = (gwa, gwb, gwrow)[mc]
                    nc.vector.tensor_copy(dst[:mlen, :], gps)

                # ========== unpack C_ext / E_ext ==========================
                crect = sb.tile([128, PAY], BF, tag="crect", bufs=2)
                nc.sync.dma_start(crect[:], c_out[:, :])
                cga = crect[:, 0:LOWE]
                ega = crect[:, PACK:PACK + LOWE]
                cgb = sb.tile([128, LOWE], BF, tag="cgb", bufs=2)
                nc.gpsimd.tensor_copy(cgb[:, 128:LOWE], crect[:, LOWE:PACK])
                egb = sb.tile([128, LOWE], BF, tag="egb", bufs=2)
                nc.gpsimd.tensor_copy(egb[:, 128:LOWE],
                                      crect[:, PACK + LOWE:PAY])
                # mirror blocks: rows 128:256 cols 0:128 = block(0,1)^T
                tm = ps.tile([128, HC, 128], BF, tag="tpb", bufs=2)
                nc.tensor.transpose(tm[:, 0, :], cga[:, 128:256], ident_b)
                nc.tensor.transpose(tm[:, 1, :], ega[:, 128:256], ident_b)
                nc.vector.tensor_copy(cgb[:, 0:128], tm[:, 0, :])
                nc.vector.tensor_copy(egb[:, 0:128], tm[:, 1, :])
                # s rows: row 256 = [s^T | BN] from the s columns
                srow = sb.tile([1, LOWE], BF, tag="srow", bufs=2)
                erow = sb.tile([1, LOWE], BF, tag="erow", bufs=2)
                tr1 = ps.tile([128, HC, 128], BF, tag="tpb", bufs=2)
                nc.tensor.transpose(tr1[0:1, 0, :], cga[:, 256:LOWE], ident_b)
                nc.tensor.transpose(tr1[0:1, 1, :], cgb[:, 256:LOWE], ident_b)
                tr2 = ps.tile([128, HC, 128], BF, tag="tpb", bufs=2)
                nc.tensor.transpose(tr2[0:1, 0, :], ega[:, 256:LOWE], ident_b)
                nc.tensor.transpose(tr2[0:1, 1, :], egb[:, 256:LOWE], ident_b)
                nc.vector.tensor_copy(srow[:, 0:256], tr1[0:1, 0:2, :])
                nc.gpsimd.memset(srow[:, 256:LOWE], float(BN))
                nc.vector.tensor_copy(erow[:, 0:256], tr2[0:1, 0:2, :])
                nc.gpsimd.memset(erow[:, 256:LOWE], float(BN))

                # ========== T2 = C_ext @ Gw  (257 x 512) ==================
                cg_tiles = [cga, cgb, srow]
                eg_tiles = [ega, egb, erow]
                gw_tiles = [gwa, gwb, gwrow]
                t2a = sb.tile([128, HIGH], BF, tag="t2a")
                t2b = sb.tile([128, HIGH], BF, tag="t2b")
                t2row = sb.tile([1, HIGH], BF, tag="t2row")
                t2_tiles = [t2a, t2b, t2row]
                for mc in range(3):
                    msl = (slice(0, 128), slice(128, 256),
                           slice(256, 257))[mc]
                    mlen = msl.stop - msl.start
                    t2f = ps.tile([128, 512], FP, tag="mm", bufs=4)
                    t2ps = t2f[:mlen, :]
                    for k in range(3):
                        klen = 128 if k < 2 else 1
                        nc.tensor.matmul(t2ps, cg_tiles[k][:klen, msl],
                                         gw_tiles[k][:klen, :],
                                         start=(k == 0), stop=(k == 2))
                    nc.vector.tensor_copy(t2_tiles[mc][:mlen, :], t2ps)

                # ========== V = P_ext^T @ T2 / BN  (256 x 512) ============
                v = sb.tile([128, LOW // 128, HIGH], BF, tag="v", bufs=2)
                v_f = sb.tile([128, LOW // 128, HIGH], FP, tag="v_f")
                for ac in range(LOW // 128):
                    vps = ps.tile([128, 512], FP, tag="mm", bufs=4)
                    for k in range(3):
                        klen = 128 if k < 2 else 1
                        nc.tensor.matmul(
                            vps, pext_b[:klen, k, ac * 128:(ac + 1) * 128],
                            t2_tiles[k][:klen, :],
                            start=(k == 0), stop=(k == 2))
                    nc.vector.tensor_scalar_mul(v[:, ac, :], vps, 1.0 / BN)
                    nc.scalar.activation(v_f[:, ac, :], vps,
                                         mybir.ActivationFunctionType.Copy,
                                         scale=1.0 / BN)

                # ---- ve_row: c^T = theta_b^T V   (1 x 512)
                ve_row = sb.tile([1, HIGH], BF, tag="ve_row")
                ve_row_f = sb.tile([1, HIGH], FP, tag="ve_row_f")
                cps2 = ps.tile([1, 512], FP, tag="mm", bufs=4)
                for k in range(LOW // 128):
                    nc.tensor.matmul(cps2, thb_b[:, k:k + 1], v[:, k, :],
                                     start=(k == 0),
                                     stop=(k == LOW // 128 - 1))
                nc.vector.tensor_copy(ve_row[:], cps2)
                nc.vector.tensor_copy(ve_row_f[:], cps2)

                # ========== W1 = E_ext @ Ve  (257 x 512) ==================
                ve_tiles = [v[:, 0, :], v[:, 1, :], ve_row]
                w1_row = sb.tile([1, HIGH], FP, tag="w1_row")
                # prod = Ve .* W1 multiplied straight out of the W1 PSUM
                prod = sb.tile([128, 2, HIGH], FP, tag="prod")
                prod_row = sb.tile([1, HIGH], FP, tag="prod_row")
                for mc in range(3):
                    msl = (slice(0, 128), slice(128, 256),
                           slice(256, 257))[mc]
                    mlen = msl.stop - msl.start
                    wf = ps.tile([128, 512], FP, tag="mm", bufs=4)
                    wps = wf[:mlen, :]
                    for k in range(3):
                        klen = 128 if k < 2 else 1
                        nc.tensor.matmul(wps, eg_tiles[k][:klen, msl],
                                         ve_tiles[k][:klen, :],
                                         start=(k == 0), stop=(k == 2))
                    if mc < 2:
                        nc.vector.tensor_mul(prod[:, mc, :], v_f[:, mc, :],
                                             wps)
                    else:
                        nc.vector.tensor_copy(w1_row[:], wps)
                        nc.vector.tensor_mul(prod_row[:], ve_row_f[:], wps)

                # ========== w_y^T GEMM groups (PE) ========================
                zt = sb.tile([128, HC, TPC], FP, tag="zt", bufs=2)

                def wy_group(nn, hc):
                    wps = ps.tile([128, 512], FP, tag="mm", bufs=4)
                    for k in range(LOW // 128):
                        nc.tensor.matmul(
                            wps, v[:, k, hc * 128:(hc + 1) * 128],
                            tht[:, k, nn * 512:(nn + 1) * 512],
                            start=(k == 0), stop=(k == LOW // 128 - 1))
                    return wps

                def apply_bn(nn, hc, wps):
                    # zt = wps * A + D; Act and DVE alternate by hc
                    if hc % 2 == 0:
                        nc.scalar.activation(
                            zt[:, hc, nn * 512:(nn + 1) * 512], wps,
                            mybir.ActivationFunctionType.Identity,
                            bias=d_p[:, hc:hc + 1], scale=a_p[:, hc:hc + 1])
                    else:
                        nc.vector.tensor_scalar(
                            zt[:, hc, nn * 512:(nn + 1) * 512], wps,
                            a_p[:, hc:hc + 1], d_p[:, hc:hc + 1],
                            mybir.AluOpType.mult, mybir.AluOpType.add)

                def z_store(i):
                    tpz = ps.tile([128, HC, 128], FP, tag="tp", bufs=2)
                    for hc in range(HC):
                        nc.tensor.transpose(
                            tpz[:, hc, :], zt[:, hc, i * 128:(i + 1) * 128],
                            ident)
                    z_sb = sb.tile([128, HIGH], FP, tag="z_sb", bufs=4)
                    nc.vector.tensor_add(
                        z_sb[:], tpz[:].rearrange("p a b -> p (a b)"),
                        xh[:, i, :])
                    q = nc.sync if i % 2 == 0 else nc.scalar
                    q.dma_start(z_out[i * 128:(i + 1) * 128, :], z_sb[:])

                # PE starts the first token group's GEMMs while DVE computes
                # prod / Q / S and the BN scalar chain.
                wps0 = [wy_group(0, hc) for hc in range(HC)]

                # ---- Q[h] = sum_k prod[k, h] and mean -> [128, HC]
                # feature-major N=1 matmuls; ones_c carries the 1/BN scale,
                # so qs[:,0] = E[w_y^2] and qs[:,1] = mean directly.
                qs = ps.tile([128, HC, 128], FP, tag="tp", bufs=2)
                for fc in range(HC):
                    nc.tensor.matmul(qs[:, 0, fc:fc + 1],
                                     prod[:, 0, fc * 128:(fc + 1) * 128],
                                     ones_c[:], start=True, stop=False)
                    nc.tensor.matmul(qs[:, 0, fc:fc + 1],
                                     prod[:, 1, fc * 128:(fc + 1) * 128],
                                     ones_c[:], start=False, stop=False)
                    nc.tensor.matmul(qs[:, 0, fc:fc + 1],
                                     prod_row[0:1, fc * 128:(fc + 1) * 128],
                                     ones_c[0:1, :], start=False, stop=True)
                for fc in range(HC):
                    nc.tensor.matmul(qs[:, 1, fc:fc + 1],
                                     w1_row[0:1, fc * 128:(fc + 1) * 128],
                                     ones_c[0:1, :], start=True, stop=True)

                # ========== BN scalars (overlap the w_y GEMM) =============
                # var = E[x^2] - mean^2; A = gamma/sqrt(var+eps);
                # D = beta - mean*gamma/sqrt(var+eps)
                mean_p = sb.tile([128, HC], FP, tag="mean_p")
                nc.vector.tensor_copy(mean_p[:], qs[:, 1, 0:HC])
                msq_p = sb.tile([128, HC], FP, tag="msq_p")
                nc.vector.tensor_mul(msq_p[:], mean_p[:], qs[:, 1, 0:HC])
                mg_p = sb.tile([128, HC], FP, tag="mg_p")
                nc.vector.tensor_mul(mg_p[:], mean_p[:], gamma_p[:])
                var_p = sb.tile([128, HC], FP, tag="var_p")
                nc.vector.tensor_sub(var_p[:], qs[:, 0, 0:HC], msq_p[:])
                std_p = sb.tile([128, HC], FP, tag="std_p")
                nc.scalar.activation(std_p[:], var_p[:],
                                     mybir.ActivationFunctionType.Sqrt,
                                     bias=eps_c[:])
                nc.vector.reciprocal(std_p[:], std_p[:])
                a_p = sb.tile([128, HC], FP, tag="a_p", bufs=2)
                nc.vector.tensor_mul(a_p[:], gamma_p[:], std_p[:])
                ma_p = sb.tile([128, HC], FP, tag="ma_p")
                nc.vector.tensor_mul(ma_p[:], mg_p[:], std_p[:])
                d_p = sb.tile([128, HC], FP, tag="d_p", bufs=2)
                nc.vector.tensor_sub(d_p[:], beta_p[:], ma_p[:])

                # ========== fused BN apply + transpose/residual/store =====
                # nn-major: all HC chunks of a 512-token group finish
                # together, so group nn's store tail overlaps group nn+1's
                # GEMM + apply.
                for hc in range(HC):
                    apply_bn(0, hc, wps0[hc])
                for j in range(4):
                    z_store(j)
                for nn in range(1, TPC // 512):
                    for hc in range(HC):
                        apply_bn(nn, hc, wy_group(nn, hc))
                    for j in range(4):
                        z_store(nn * 4 + j)

    nc.compile()
    return nc


_CACHE: dict[int, "bacc.Bacc"] = {}
_LOCK = threading.Lock()


def _get_nc(repeats: int = 1):
    with _LOCK:
        if repeats not in _CACHE:
            _CACHE[repeats] = build_kernel(repeats)
        return _CACHE[repeats]


def _shard_inputs(inputs: dict) -> list[dict]:
    xh = np.ascontiguousarray(
        np.asarray(inputs["x_h"], dtype=np.float32).reshape(BN, HIGH))
    xl = np.ascontiguousarray(
        np.asarray(inputs["x_l"], dtype=np.float32).reshape(BN, LOW))
    # w_b is intentionally unused: BatchNorm output is invariant to a
    # constant shift of its input, so the w_b add cancels exactly.
    common = {
        "g_w": np.asarray(inputs["g_w"], np.float32),
        "g_b": np.asarray(inputs["g_b"], np.float32),
        "theta_w": np.asarray(inputs["theta_w"], np.float32),
        "theta_b": np.asarray(inputs["theta_b"], np.float32),
        "phi_w": np.asarray(inputs["phi_w"], np.float32),
        "phi_b": np.asarray(inputs["phi_b"], np.float32),
        "w_w": np.asarray(inputs["w_w"], np.float32),
        "bn_gamma": np.asarray(inputs["bn_gamma"], np.float32),
        "bn_beta": np.asarray(inputs["bn_beta"], np.float32),
    }
    return [
        {"x_h": xh[c * TPC:(c + 1) * TPC],
         "x_l": xl[c * TPC:(c + 1) * TPC], **common}
        for c in range(NCORES)
    ]


def kernel(**inputs) -> np.ndarray:
    nc = _get_nc(1)
    in_maps = _shard_inputs(inputs)
    res = run_bass_kernel_spmd(nc, in_maps, list(range(NCORES)))
    z = np.concatenate([res.results[c]["z"] for c in range(NCORES)], axis=0)
    return z.reshape(B, N, HIGH)
